# revision 11
# baseline (speedup 1.0000x reference)
"""Trainium2 Bass kernel for nn_ConstraintLayer (feasibility-projection layer).

Reference computation (B=4096, IN=2048, N=512, K=1024, NQ=8):
    qm = x @ W_map.T + b_map            -> v = qm[:, :N], beta = qm[:, N]
    v_bar = v / max(||v||, 1e-12)
    kappa_lin = relu(max_j (v_bar @ D.T)_j)
    rho = v_bar @ NA_E.T
    a_i = 0.5 rho^T P_i rho ; bq_i = rho . (P_i w + q_i) ; c_i consts
    lam_i = (-bq + sqrt(bq^2 - 4 a c)) / (2a)
    kappa = max(kappa_lin, max_i 1/lam_i)
    alpha = 1/(exp(beta) + kappa)
    y = (z0 + alpha v_bar) @ NA_E.T + y1

Strategy: pure batch data-parallel over 8 cores (512 rows each).  Row
normalization is folded into per-row scalars so every matmul runs on raw v:
    s_raw  = rho_raw^T P_i rho_raw        (rho_raw = v @ NA_E.T)
    disc   = bq_raw^2 - 2 s_raw c
    invlam_raw = s_raw / (sqrt(disc) - bq_raw)
    kappa_raw  = max(max_j (v @ D.T)_j, 0, max_i invlam_raw)
    s = 1/(vn exp(beta) + kappa_raw)      (vn = max(||v||, 1e-12))
    y = s * rho_raw + (NA_E z0 + y1)

v2 kernel changes vs the f32r baseline:
  * s_raw via Cholesky: P_i = L_i L_i^T, s_raw = ||rho_raw @ L_i||^2.  The
    T = rho8 @ L8 matmuls run in fp8e4 with DoubleRow perf mode (2x PE rate,
    256-deep contraction), and L lower-triangular kills the (k<512, col>=512)
    quarter: 6 passes instead of 8 per (i, batch-tile).  The row-sum of T^2
    is a single ScalarE Square pass with accum_out per PSUM bank -- no more
    DVE multiply+reduce (was ~68us of Vector time).
  * all other matmul operands in bf16 (same PE rate as f32r, half the DMA).
  * ~26us serial DMA head fixed: DMA dispatches spread across engine queues
    (sync/scalar: xT, vector: WT, scalar: NA/D, gpsimd: L8 + small consts)
    and the PE warms the HAM clock gate with dummy matmuls during the head.
  * w broadcast [K] -> [128, K] built with a ones-matmul on the PE instead of
    a 128-row gpsimd software-DGE DMA (was ~8us).
Measured numerics: rel err ~3e-3 vs fp32 reference (gate 2e-2).
"""

import numpy as np

import concourse.bass as bass
import concourse.mybir as mybir
import concourse.tile as tile
from concourse import bacc
from concourse.masks import make_identity

F32 = mybir.dt.float32
F32R = mybir.dt.float32r
BF16 = mybir.dt.bfloat16
FP8 = mybir.dt.float8e4

B = 4096
IN = 2048
N = 512
K = 1024
NQ = 8
NCORES = 8
BC = B // NCORES          # 512 batch rows per core
P128 = 128
NB_IN = IN // P128        # 16
NB_N = N // P128          # 4
NB_K = K // P128          # 8
NB_B = BC // P128         # 4
KC = K // 256             # 4 DoubleRow contraction chunks
KH = 512                  # psum-bank-sized free dim

AX = mybir.AxisListType
ALU = mybir.AluOpType
ACTF = mybir.ActivationFunctionType
DR = mybir.MatmulPerfMode.DoubleRow


def _build():
    nc = bacc.Bacc()

    xt_d = nc.dram_tensor("xT", [IN, BC], BF16, kind="ExternalInput")
    wt_d = nc.dram_tensor("WT", [IN, N + 1], BF16, kind="ExternalInput")
    nat_d = nc.dram_tensor("NAT", [N, K], BF16, kind="ExternalInput")
    dt_d = nc.dram_tensor("DT", [N, 2 * N], BF16, kind="ExternalInput")
    l8_d = nc.dram_tensor("L8", [NQ, KC, P128, 2, K], FP8, kind="ExternalInput")
    pwq_d = nc.dram_tensor("PWQ", [P128, NB_K * NQ], BF16, kind="ExternalInput")
    wrow_d = nc.dram_tensor("WROW", [1, K], BF16, kind="ExternalInput")
    c_d = nc.dram_tensor("CVEC", [NQ], F32, kind="ExternalInput")
    b_d = nc.dram_tensor("BPACK", [P128, NB_N + 1], F32, kind="ExternalInput")
    y_d = nc.dram_tensor("Y", [BC, K], F32, kind="ExternalOutput")

    with tile.TileContext(nc) as tc:
        with (
            tc.tile_pool(name="singles", bufs=1) as singles,
            tc.tile_pool(name="persist", bufs=1) as persist,
            tc.tile_pool(name="l8s", bufs=2) as l8s,
            tc.tile_pool(name="l8b", bufs=2) as l8b,
            tc.tile_pool(name="scratch", bufs=2) as scratch,
            tc.tile_pool(name="ypool", bufs=2) as ypool,
        ):
            # ---- constants (gpsimd queue: identities first, then small DMAs,
            # then the 32 L8 chunk DMAs which stream through stage 7) ----
            identb = singles.tile([P128, P128], BF16, name="identb")
            make_identity(nc, identb[:, :])
            identf = singles.tile([P128, P128], F32, name="identf")
            make_identity(nc, identf[:, :])
            onesb = singles.tile([P128, 1], BF16, name="onesb")
            nc.vector.memset(onesb, 1.0)
            onesrow = singles.tile([1, P128], BF16, name="onesrow")
            nc.vector.memset(onesrow, 1.0)
            warm = singles.tile([P128, KH], BF16, name="warm")
            nc.vector.memset(warm, 0.0)
            wrow = singles.tile([1, K], BF16, name="wrow")
            nc.gpsimd.dma_start(out=wrow, in_=wrow_d[:, :])
            c128 = singles.tile([P128, NQ], F32, name="c128")
            nc.gpsimd.dma_start(out=c128, in_=bass.AP(c_d, 0, [[0, P128], [1, NQ]]))
            bpack = singles.tile([P128, NB_N + 1], F32, name="bpack")
            nc.gpsimd.dma_start(out=bpack, in_=b_d[:, :])
            pwqpk = singles.tile([P128, NB_K * NQ], BF16, name="pwqpk")
            nc.gpsimd.dma_start(out=pwqpk, in_=pwq_d[:, :])
            # L8 prefetch: kc0/kc1 only carry cols < 512 (L lower-triangular)
            l8t = []
            for i in range(NQ):
                t0 = l8s.tile([P128, 2, KH], FP8, tag="l8k0", name=f"l8_{i}_0")
                nc.gpsimd.dma_start(out=t0, in_=l8_d[i, 0, :, :, 0:KH])
                t1 = l8s.tile([P128, 2, KH], FP8, tag="l8k1", name=f"l8_{i}_1")
                nc.gpsimd.dma_start(out=t1, in_=l8_d[i, 1, :, :, 0:KH])
                t2 = l8b.tile([P128, 2, K], FP8, tag="l8k2", name=f"l8_{i}_2")
                nc.gpsimd.dma_start(out=t2, in_=l8_d[i, 2, :, :, :])
                t3 = l8b.tile([P128, 2, K], FP8, tag="l8k3", name=f"l8_{i}_3")
                nc.gpsimd.dma_start(out=t3, in_=l8_d[i, 3, :, :, :])
                l8t.append((t0, t1, t2, t3))

            # ---- persistent intermediates ----
            vt = [persist.tile([P128, BC], BF16, tag=f"vt{i}", name=f"vt{i}")
                  for i in range(NB_N)]
            statsT = persist.tile([P128, BC], F32, tag="statsT", name="statsT")
            nc.vector.memset(statsT, 0.0)
            rt = [persist.tile([P128, BC], BF16, tag=f"rt{i}", name=f"rt{i}")
                  for i in range(NB_K)]
            rt8 = [persist.tile([P128, 2, BC], FP8, tag=f"rt8_{i}",
                                name=f"rt8_{i}") for i in range(KC)]
            rb = [persist.tile([P128, K], F32, tag=f"rb{i}", name=f"rb{i}")
                  for i in range(NB_B)]
            sb = [persist.tile([P128, 2], F32, tag=f"sb{i}", name=f"sb{i}")
                  for i in range(NB_B)]
            amat = [persist.tile([P128, NQ], F32, tag=f"am{i}", name=f"am{i}")
                    for i in range(NB_B)]
            bqm = [persist.tile([P128, NQ], F32, tag=f"bq{i}", name=f"bq{i}")
                   for i in range(NB_B)]
            mdv = [persist.tile([P128, 1], F32, tag=f"mdv{i}", name=f"mdv{i}")
                   for i in range(NB_B)]
            ve = [persist.tile([P128, 1], F32, tag=f"ve{i}", name=f"ve{i}")
                  for i in range(NB_B)]

            # ---- stage 1: PE warmup + mapper qm^T[col, b] ----
            with (
                tc.tile_pool(name="s1sb", bufs=5) as s1sb,
                tc.tile_pool(name="s1ps", bufs=1, space="PSUM") as s1ps,
            ):
                wps = s1ps.tile([P128, KH], F32, tag="warm", name="warmps",
                                bufs=1)
                for _ in range(8):
                    nc.tensor.matmul(wps[:, :], identb[:, :], warm[:, :],
                                     start=True, stop=True)
                mm_ps = [s1ps.tile([P128, BC], F32, tag=f"map{cb}",
                                   name=f"map{cb}", bufs=1)
                         for cb in range(NB_N)]
                beta_ps = s1ps.tile([1, BC], F32, tag="mapbeta", name="mapbeta",
                                    bufs=1)
                for ib in range(NB_IN):
                    xt_t = s1sb.tile([P128, BC], BF16, tag="xt", name="xt",
                                     bufs=5)
                    nc.sync.dma_start(out=xt_t,
                                      in_=xt_d[ib * P128:(ib + 1) * P128, :])
                    wt_t = s1sb.tile([P128, N + 1], BF16, tag="wt", name="wt",
                                     bufs=5)
                    nc.scalar.dma_start(out=wt_t,
                                        in_=wt_d[ib * P128:(ib + 1) * P128, :])
                    st = dict(start=(ib == 0), stop=(ib == NB_IN - 1))
                    for cb in range(NB_N):
                        nc.tensor.matmul(
                            mm_ps[cb][:, :], wt_t[:, cb * P128:(cb + 1) * P128],
                            xt_t[:, :], **st,
                        )
                    nc.tensor.matmul(beta_ps[:, :], wt_t[:, N:N + 1],
                                     xt_t[:, :], **st)
                for cb in range(NB_N):
                    nc.vector.tensor_scalar_add(
                        out=vt[cb][:, :], in0=mm_ps[cb][:, :],
                        scalar1=bpack[:, cb:cb + 1],
                    )
                nc.vector.tensor_scalar_add(
                    out=statsT[0:1, :], in0=beta_ps[:, :],
                    scalar1=bpack[0:1, NB_N:NB_N + 1],
                )

            # NA^T / D^T loads: same DMA ring as xT (sync), dispatched after
            # all 16 xT tiles -- ring FIFO order keeps them from stealing HBM
            # bandwidth from the mapper's last tiles
            nat = []
            for nb in range(NB_N):
                t = singles.tile([P128, K], BF16, tag=f"nat{nb}",
                                 name=f"nat{nb}")
                nc.sync.dma_start(out=t,
                                  in_=nat_d[nb * P128:(nb + 1) * P128, :])
                nat.append(t)
            dmat = []
            for nb in range(NB_N):
                t = singles.tile([P128, 2 * N], BF16, tag=f"dt{nb}",
                                 name=f"dt{nb}")
                nc.sync.dma_start(out=t,
                                  in_=dt_d[nb * P128:(nb + 1) * P128, :])
                dmat.append(t)

            with tc.tile_pool(name="ps2", bufs=1, space="PSUM") as ps2:
                # ---- stage 2: vn^2 via ones-matmul over squared v^T ----
                vn2_ps = ps2.tile([1, BC], F32, tag="vn2", name="vn2ps", bufs=1)
                for nb in range(NB_N):
                    sq = scratch.tile([P128, BC], BF16, tag="sq", name="sq",
                                      bufs=2)
                    nc.vector.tensor_mul(out=sq[:, :], in0=vt[nb][:, :],
                                         in1=vt[nb][:, :])
                    nc.tensor.matmul(
                        vn2_ps[:, :], onesb[:, :], sq[:, :],
                        start=(nb == 0), stop=(nb == NB_N - 1),
                    )
                nc.vector.tensor_copy(out=statsT[32:33, :], in_=vn2_ps[:, :])

                # ---- w broadcast [K] -> [128, K] via ones-matmul ----
                w128 = singles.tile([P128, K], F32, name="w128")
                for kh in range(2):
                    wbc = ps2.tile([P128, KH], F32, tag="wbc", name="wbc",
                                   bufs=1)
                    nc.tensor.matmul(wbc[:, :], onesrow[:, :],
                                     wrow[0:1, kh * KH:(kh + 1) * KH],
                                     start=True, stop=True)
                    nc.vector.tensor_copy(out=w128[:, kh * KH:(kh + 1) * KH],
                                          in_=wbc[:, :])

                # ---- stage 3: rho^T[k, b] + bf16/fp8 shadows ----
                for kb in range(NB_K):
                    mm = ps2.tile([P128, KH], F32, tag="mm", name="mm", bufs=3)
                    for nb in range(NB_N):
                        nc.tensor.matmul(
                            mm[:, :], nat[nb][:, kb * P128:(kb + 1) * P128],
                            vt[nb][:, :],
                            start=(nb == 0), stop=(nb == NB_N - 1),
                        )
                    nc.scalar.copy(out=rt[kb][:, :], in_=mm[:, :])
                    nc.scalar.copy(out=rt8[kb // 2][:, kb % 2, :], in_=mm[:, :])

                # ---- stage 4: transposes -> rho[b, k]; per-row stats ----
                for bb in range(NB_B):
                    pst = ps2.tile([P128, P128], F32, tag="tr", name="tr",
                                   bufs=1)
                    nc.tensor.transpose(
                        pst[:, :], statsT[:, bb * P128:(bb + 1) * P128],
                        identf[:, :],
                    )
                    nc.vector.tensor_copy(out=sb[bb][:, 0:1], in_=pst[:, 0:1])
                    nc.vector.tensor_copy(out=sb[bb][:, 1:2], in_=pst[:, 32:33])
                for kb in range(NB_K):
                    for bb in range(NB_B):
                        pstb = ps2.tile([P128, P128], BF16, tag="trb",
                                        name="trb", bufs=2)
                        nc.tensor.transpose(
                            pstb[:, :], rt[kb][:, bb * P128:(bb + 1) * P128],
                            identb[:, :],
                        )
                        nc.scalar.copy(
                            out=rb[bb][:, kb * P128:(kb + 1) * P128],
                            in_=pstb[:, :],
                        )
                # per-row scalars: ve2 = vn^2 * exp(2 beta); sqrt deferred to
                # stage 8 so the ACT Sqrt table loads once (1e-12 clamp dropped:
                # ||v|| ~ 22 for this layer, never near zero)
                for bb in range(NB_B):
                    e_t = scratch.tile([P128, 1], F32, tag="e", name="e",
                                       bufs=2)
                    nc.scalar.activation(out=e_t[:, :], in_=sb[bb][:, 0:1],
                                         func=ACTF.Exp, scale=2.0)
                    nc.vector.tensor_mul(out=ve[bb][:, :], in0=sb[bb][:, 1:2],
                                         in1=e_t[:, :])

                # ---- stage 5: Dv; kappa_lin(raw) = max_j ----
                for bb in range(NB_B):
                    hmx = scratch.tile([P128, 2], F32, tag="hmx", name="hmx",
                                       bufs=2)
                    for kh in range(2):
                        mm = ps2.tile([P128, KH], F32, tag="mm", name="mm",
                                      bufs=3)
                        for nb in range(NB_N):
                            nc.tensor.matmul(
                                mm[:, :], vt[nb][:, bb * P128:(bb + 1) * P128],
                                dmat[nb][:, kh * KH:(kh + 1) * KH],
                                start=(nb == 0), stop=(nb == NB_N - 1),
                            )
                        nc.vector.tensor_reduce(
                            out=hmx[:, kh:kh + 1], in_=mm[:, :], axis=AX.X,
                            op=ALU.max,
                        )
                    nc.vector.tensor_reduce(
                        out=mdv[bb][:, :], in_=hmx[:, :], axis=AX.X, op=ALU.max
                    )

            # ---- stage 6 + 7 ----
            with tc.tile_pool(name="ps7", bufs=1, space="PSUM") as ps7:
                # stage 6: bq[b, i]
                for bb in range(NB_B):
                    mmq = ps7.tile([P128, NQ], F32, tag="mmq", name="mmq",
                                   bufs=2)
                    for kb in range(NB_K):
                        nc.tensor.matmul(
                            mmq[:, :], rt[kb][:, bb * P128:(bb + 1) * P128],
                            pwqpk[:, kb * NQ:(kb + 1) * NQ],
                            start=(kb == 0), stop=(kb == NB_K - 1),
                        )
                    nc.vector.tensor_copy(out=bqm[bb][:, :], in_=mmq[:, :])

                # stage 7: T = rho8 @ L8_i (fp8 DoubleRow), s_raw = sum T^2
                for i in range(NQ):
                    t0, t1, t2, t3 = l8t[i]
                    for bb in range(NB_B):
                        bbs = slice(bb * P128, (bb + 1) * P128)
                        ps = ps7.tile([P128, K], F32, tag="s7", name="s7",
                                      bufs=3)
                        psA = ps[:, 0:KH]
                        psB = ps[:, KH:K]
                        nc.tensor.matmul(psA, rt8[0][:, :, bbs],
                                         t0[:, :, :], start=True, stop=False,
                                         perf_mode=DR)
                        nc.tensor.matmul(psA, rt8[1][:, :, bbs],
                                         t1[:, :, :], start=False, stop=False,
                                         perf_mode=DR)
                        nc.tensor.matmul(psA, rt8[2][:, :, bbs],
                                         t2[:, :, 0:KH], start=False,
                                         stop=False, perf_mode=DR)
                        nc.tensor.matmul(psB, rt8[2][:, :, bbs],
                                         t2[:, :, KH:K], start=True,
                                         stop=False, perf_mode=DR)
                        nc.tensor.matmul(psA, rt8[3][:, :, bbs],
                                         t3[:, :, 0:KH], start=False,
                                         stop=True, perf_mode=DR)
                        nc.tensor.matmul(psB, rt8[3][:, :, bbs],
                                         t3[:, :, KH:K], start=False,
                                         stop=True, perf_mode=DR)
                        junk = scratch.tile([P128, K], F32, tag="junk",
                                            name="junk", bufs=2)
                        nc.scalar.activation(out=junk[:, :], in_=ps[:, :],
                                             func=ACTF.Square,
                                             accum_out=amat[bb][:, i:i + 1])

            # ---- stage 8: per-row finale + y ----
            for bb in range(NB_B):
                t8 = scratch.tile([P128, NQ], F32, tag="t8", name="t8", bufs=2)
                nc.vector.tensor_mul(out=t8[:, :], in0=bqm[bb][:, :],
                                     in1=bqm[bb][:, :])
                t9 = scratch.tile([P128, NQ], F32, tag="t9", name="t9", bufs=2)
                nc.vector.tensor_mul(out=t9[:, :], in0=amat[bb][:, :],
                                     in1=c128[:, :])
                disc = scratch.tile([P128, NQ], F32, tag="disc", name="disc",
                                    bufs=2)
                # disc = bq^2 - 2 s_raw c
                nc.vector.scalar_tensor_tensor(
                    out=disc[:, :], in0=t9[:, :], scalar=-2.0, in1=t8[:, :],
                    op0=ALU.mult, op1=ALU.add,
                )
                nc.scalar.activation(out=disc[:, :], in_=disc[:, :],
                                     func=ACTF.Sqrt)
                nc.vector.tensor_sub(out=disc[:, :], in0=disc[:, :],
                                     in1=bqm[bb][:, :])
                nc.vector.reciprocal(out=disc[:, :], in_=disc[:, :])
                nc.vector.tensor_mul(out=disc[:, :], in0=disc[:, :],
                                     in1=amat[bb][:, :])
                ilm = scratch.tile([P128, 1], F32, tag="ilm", name="ilm",
                                   bufs=2)
                nc.vector.tensor_reduce(
                    out=ilm[:, :], in_=disc[:, :], axis=AX.X, op=ALU.max
                )
                sve = scratch.tile([P128, 1], F32, tag="sve", name="sve",
                                   bufs=2)
                nc.scalar.activation(out=sve[:, :], in_=ve[bb][:, :],
                                     func=ACTF.Sqrt)
                kap = scratch.tile([P128, 1], F32, tag="kap", name="kap",
                                   bufs=2)
                nc.vector.tensor_scalar_max(out=kap[:, :], in0=mdv[bb][:, :],
                                            scalar1=0.0)
                nc.vector.tensor_max(out=kap[:, :], in0=kap[:, :],
                                     in1=ilm[:, :])
                nc.vector.tensor_add(out=kap[:, :], in0=kap[:, :],
                                     in1=sve[:, :])
                nc.vector.reciprocal(out=kap[:, :], in_=kap[:, :])
                yt = ypool.tile([P128, K], F32, tag="y", name="yt")
                nc.vector.scalar_tensor_tensor(
                    out=yt[:, :], in0=rb[bb][:, :], scalar=kap[:, :],
                    in1=w128[:, :], op0=ALU.mult, op1=ALU.add,
                )
                nc.sync.dma_start(out=y_d[bb * P128:(bb + 1) * P128, :],
                                  in_=yt[:, :])

    nc.compile()
    return nc


_NC_CACHE = {}


def _get_nc():
    if "nc" not in _NC_CACHE:
        _NC_CACHE["nc"] = _build()
    return _NC_CACHE["nc"]


def _prepare_host(inputs):
    import ml_dtypes

    BF = ml_dtypes.bfloat16
    E4 = ml_dtypes.float8_e4m3

    f = lambda a: np.asarray(a, dtype=np.float32)
    x = f(inputs["x"])
    W_map = f(inputs["W_map"])
    b_map = f(inputs["b_map"])
    D = f(inputs["D"])
    NA_E = f(inputs["NA_E"])
    y1 = f(inputs["y1"])
    z0 = f(inputs["z0"])
    all_P = f(inputs["all_P"])
    all_q = f(inputs["all_q"])
    all_r = f(inputs["all_r"])

    WT = np.ascontiguousarray(W_map.T.astype(BF))           # [IN, N+1]
    NAT = np.ascontiguousarray(NA_E.T.astype(BF))           # [N, K]
    DT = np.ascontiguousarray(D.T.astype(BF))               # [N, 2N]
    w = (NA_E @ z0 + y1)[:, 0]                              # [K]
    Pw = np.einsum("ikj,j->ik", all_P, w) + all_q[:, :, 0]  # [NQ, K]
    cv = (
        0.5 * np.einsum("k,ikj,j->i", w, all_P, w)
        + all_q[:, :, 0] @ w
        + all_r[:, 0, 0]
    )
    pwq = np.ascontiguousarray(
        Pw.reshape(NQ, NB_K, P128).transpose(2, 1, 0)
        .reshape(P128, NB_K * NQ).astype(BF)
    )
    bpack = np.zeros((P128, NB_N + 1), np.float32)
    bpack[:, :NB_N] = b_map[:N].reshape(NB_N, P128).T
    bpack[0, NB_N] = b_map[N]

    # Cholesky factors, fp8, DoubleRow layout [i, kc, p, s, col]
    L = np.linalg.cholesky(all_P.astype(np.float64)).astype(np.float32)
    L8 = np.ascontiguousarray(
        L.reshape(NQ, KC, 2, P128, K).transpose(0, 1, 3, 2, 4).astype(E4)
    )

    shared = dict(
        WT=WT, NAT=NAT, DT=DT, L8=L8, PWQ=pwq,
        WROW=np.ascontiguousarray(w.astype(BF).reshape(1, K)),
        CVEC=np.ascontiguousarray(cv.astype(np.float32)),
        BPACK=bpack,
    )
    in_maps = []
    for c in range(NCORES):
        m = dict(shared)
        m["xT"] = np.ascontiguousarray(x[c * BC:(c + 1) * BC, :].T.astype(BF))
        in_maps.append(m)
    return in_maps


def kernel(**inputs) -> np.ndarray:
    from concourse.bass_utils import run_bass_kernel_spmd

    in_maps = _prepare_host(inputs)
    nc = _get_nc()
    res = run_bass_kernel_spmd(nc, in_maps, core_ids=list(range(NCORES)))
    return np.concatenate([res.results[c]["Y"] for c in range(NCORES)], axis=0)
